# revision 2
# baseline (speedup 1.0000x reference)
"""Distributed Trainium2 Bass kernel for nn_ActivationAndBiophysModel.

2-layer GRU (H=512) + FC + muscle biophysics, T=512, B=64, on 8 cores.

Architecture (SPMD: all cores run the SAME instruction stream, data differs):
- core 0 ("A"): runs the layer-0 GRU recurrence. Its gate input gi0 =
  x@W_ih0^T + b enters via a 5th contraction chunk (stationary = padded
  x^T with a ones-row; moving = padded W_ih0 panel).
- core 1 ("B"): runs the layer-1 GRU recurrence 4 phases behind, plus FC
  and the biophysics integrator. Its gate input gi1(t) = h0(t)@W_ih1^T + b
  is injected via an identity-stationary matmul from an SBUF buffer that
  is assembled from per-core slices every phase.
- all cores: compute a 192-column slice of gi1 for a previous block from
  the AllGather'ed h0T block (the "tail" matmuls, interleaved into the
  step loop to fill PE bubbles).
- Per phase (S=16 steps): AG1 ships core0's h0T block; AG2 ships each
  core's gi1 slice. Both overlap the next phase's compute.
- Core divergence is data-driven only: selector/weight tensors are zeros
  on cores where a term must vanish; B's h1 state is frozen during the
  4 warm-up phases by feeding z-gate inputs of +40 (sigmoid -> 1.0).
- PE array usage: gate matmuls are (128,64)-tiled; outputs are split
  between PSUM partitions 0-63 (tile T0: r-gate, hn) and 64-127
  (tile T1: z-gate, n-input, gi1 slices, FC) so the two PE column-tiles
  can stream concurrently.

Output comes from core 1 (results[1]["out"]).
"""

import sys

for p in ("/opt/trn_rl_repo", "/opt/pypackages"):
    if p not in sys.path:
        sys.path.insert(0, p)

import numpy as np  # noqa: E402

B, T, IN, H, J = 64, 512, 16, 512, 8
HG = 3 * H
NC = 8
SL = HG // 4  # 384: gi1 slice width (ranks 0-3 consumed)
LAG = 4

# muscle / joint constants
K0, K1, L0m, L1m, Mm = 100.0, 2000.0, 0.06, 0.006, 0.05
Ij, Kj, Bj, DT = 0.004, 5.0, 0.3, 1.0 / 60.0
_c = DT / Ij
ALPHA = 1.0 - _c * Bj
BETA = _c * Mm * (K0 * L1m + K1 * L0m)
GAMMA = _c * Mm * K1 * L1m
DELTA = _c * (-(2.0 * Mm * Mm * K0) - Kj)
EPS = _c * (-(Mm * Mm * K1))
FREEZE = 40.0


def _build(nc, bass, tile, mybir, T_run, S=16):
    f32 = mybir.dt.float32
    bf16 = mybir.dt.bfloat16
    AF = mybir.ActivationFunctionType
    NPH = T_run // S
    NTOT = NPH + LAG

    mm = nc.tensor.matmul

    # ---- DRAM parameters (per-core contents differ) ----------------------
    whh5_d = nc.declare_dram_parameter("whh5", [6, 128, HG], bf16, isOutput=False)
    wsl_d = nc.declare_dram_parameter("wsl", [4, 128, SL], bf16, isOutput=False)
    wfc5_d = nc.declare_dram_parameter("wfc5", [5, 128, 2 * J], bf16, isOutput=False)
    xTp_d = nc.declare_dram_parameter("xTp", [NPH, 128, 64 * S], bf16, isOutput=False)
    selgi_d = nc.declare_dram_parameter("selgi", [128, 64], bf16, isOutput=False)
    ident_d = nc.declare_dram_parameter("ident", [64, 64], bf16, isOutput=False)
    bhn_d = nc.declare_dram_parameter("bhn", [64, H], bf16, isOutput=False)
    hb0_d = nc.declare_dram_parameter("hb0", [64, H], bf16, isOutput=False)
    hT0_d = nc.declare_dram_parameter("hT0", [128, 256], bf16, isOutput=False)
    gi1init_d = nc.declare_dram_parameter("gi1init", [64, S, HG], bf16,
                                          isOutput=False)
    th0_d = nc.declare_dram_parameter("th0", [64, J], f32, isOutput=False)
    om0_d = nc.declare_dram_parameter("om0", [64, J], f32, isOutput=False)
    out_d = nc.declare_dram_parameter("out", [64, T_run * J], f32, isOutput=True)

    RG = [[0, 1, 2, 3, 4, 5, 6, 7]]

    with tile.TileContext(nc) as tc:
        with (
            tc.tile_pool(name="wp", bufs=1) as wp,
            tc.tile_pool(name="xp", bufs=2) as xp,
            tc.tile_pool(name="hbp", bufs=2) as hbp,
            tc.tile_pool(name="sp", bufs=2) as sp,
            tc.tile_pool(name="gp", bufs=2) as gp,
            tc.tile_pool(name="ap_", bufs=2) as apl,
            tc.tile_pool(name="bp", bufs=2) as bp,
            tc.tile_pool(name="dp", bufs=1, space="DRAM") as dp,
            tc.tile_pool(name="prz", bufs=2, space="PSUM") as prz,
            tc.tile_pool(name="ptt", bufs=2, space="PSUM") as ptt,
            tc.tile_pool(name="pax", bufs=2, space="PSUM") as pax,
        ):
            # ---- constants / weights ------------------------------------
            whh5 = wp.tile([128, 6 * HG], bf16)
            for c in range(6):
                nc.sync.dma_start(whh5[:, c * HG : (c + 1) * HG], whh5_d[c])
            wsl = wp.tile([128, 4 * SL], bf16)
            for c in range(4):
                nc.sync.dma_start(wsl[:, c * SL : (c + 1) * SL], wsl_d[c])
            wfc5 = wp.tile([128, 5 * 2 * J], bf16)
            for c in range(5):
                nc.sync.dma_start(wfc5[:, c * 2 * J : (c + 1) * 2 * J], wfc5_d[c])
            selgi = wp.tile([128, 64], bf16)
            nc.sync.dma_start(selgi[:], selgi_d[:])
            ident = wp.tile([64, 64], bf16)
            nc.sync.dma_start(ident[:], ident_d[:])
            bhn = wp.tile([64, H], bf16)
            nc.sync.dma_start(bhn[:], bhn_d[:])
            hT0 = wp.tile([128, 256], bf16)
            nc.sync.dma_start(hT0[:], hT0_d[:])

            gi1buf = wp.tile([128, S, HG], bf16)
            nc.vector.memset(gi1buf[64:128, :, :], 0.0)
            nc.sync.dma_start(gi1buf[0:64, :, :], gi1init_d[:])
            h0T_recv = wp.tile([128, 256 * S], bf16)
            nc.vector.memset(h0T_recv[:], 0.0)
            gi1sl_out_3d = wp.tile([64, S, SL], bf16)

            out_sb = wp.tile([64, T_run * J], f32)
            scratch_th = wp.tile([64, S * J], f32)

            cON = {}
            for nm, v in (("ALP", ALPHA), ("DT", DT), ("GAM", GAMMA),
                          ("BG", BETA / GAMMA), ("EPS", EPS), ("DEL", DELTA)):
                t_ = wp.tile([64, J], f32, name=f"c{nm}", tag=f"c{nm}")
                nc.gpsimd.memset(t_[:], v)
                cON[nm] = t_

            # ---- state --------------------------------------------------
            h_b = sp.tile([64, H], bf16, tag="h_b")
            nc.sync.dma_start(h_b[:], hb0_d[:])
            th_t = sp.tile([64, J], f32, tag="th")
            nc.sync.dma_start(th_t[:], th0_d[:])
            om_t = sp.tile([64, J], f32, tag="om")
            nc.sync.dma_start(om_t[:], om0_d[:])
            th_ap = th_t[:]
            om_ap = om_t[:]

            prev_hTblk = None  # [128, 256*S] tile of previous phase
            prev_ablk = None
            prev_cc1_out = None
            prev_cc2_out = None
            xts = None
            pend_fc = []  # (hTblk, slot, ablk, aslot)

            def emit_fc(src_blk, slot, ablk, aslot):
                ps_fc = pax.tile([128, 512], f32, tag="pax")
                fc = ps_fc[64:128, 0:16]
                for c in range(4):
                    st = src_blk[:, slot * 256 + c * 64 : slot * 256 + (c + 1) * 64]
                    mm(fc, st, wfc5[:, c * 16 : (c + 1) * 16],
                       start=(c == 0), stop=False)
                mm(fc, xts[:, 0:64], wfc5[:, 4 * 16 : 5 * 16],
                   start=False, stop=True)
                nc.scalar.activation(ablk[:, aslot, :, :], fc, AF.Sigmoid)
                return ps_fc

            def emit_step(p, i, hTblk, ablk, do_tail):
                nonlocal h_b
                t0 = (p == 0 and i == 0)
                if i == 0:
                    src = hT0 if t0 else prev_hTblk
                    soff = 0 if t0 else (S - 1) * 256
                else:
                    src = hTblk
                    soff = (i - 1) * 256

                grz = prz.tile([128, 1024], f32, tag="grz")
                rz_r = grz[0:64, 0:512]
                nh = grz[0:64, 512:1024]
                ni = grz[64:128, 0:512]
                rz_z = grz[64:128, 512:1024]

                # bank0: rz_r then rz_z; bank1: nh then ni (groups must not
                # overlap within a bank). Stationary-shared pairs (nh,rz_r)
                # keep LDW dedup effective.
                xt_st = xts[:, i * 64 : (i + 1) * 64]
                gsl = gi1buf[:, i, :]
                w4, w5 = 4 * HG, 5 * HG
                # hT-independent openers first (run during prev epilogue):
                # bank0: ni then rz_r; bank1: nh then rz_z.
                mm(ni, xt_st, whh5[:, w4 + 1024 : w4 + 1536],
                   start=True, stop=False)
                mm(ni, selgi[:], gsl[:, 1024:1536], start=False, stop=True)
                mm(nh, xt_st, whh5[:, w5 + 1024 : w5 + 1536],
                   start=True, stop=False)
                mm(rz_r, xt_st, whh5[:, w4 : w4 + 512], start=True, stop=False)
                mm(rz_r, selgi[:], gsl[:, 0:512], start=False, stop=False)
                for c in range(4):
                    st = src[:, soff + c * 64 : soff + (c + 1) * 64]
                    mm(nh, st, whh5[:, c * HG + 1024 : c * HG + 1536],
                       start=False, stop=(c == 3))
                    mm(rz_r, st, whh5[:, c * HG : c * HG + 512],
                       start=False, stop=(c == 3))
                mm(rz_z, xt_st, whh5[:, w4 + 512 : w4 + 1024],
                   start=True, stop=False)
                mm(rz_z, selgi[:], gsl[:, 512:1024], start=False, stop=False)
                for c in range(4):
                    st = src[:, soff + c * 64 : soff + (c + 1) * 64]
                    mm(rz_z, st, whh5[:, c * HG + 512 : c * HG + 1024],
                       start=False, stop=(c == 3))

                # independent PE work to fill the epilogue bubble
                gps = None
                if do_tail:
                    gax = pax.tile([128, 512], f32, tag="pax")
                    gps = gax[64:128, 16:16 + SL]
                    for c in range(4):
                        st = h0T_recv[:, i * 256 + c * 64 : i * 256 + (c + 1) * 64]
                        mm(gps, st, wsl[:, c * SL : (c + 1) * SL],
                           start=(c == 0), stop=(c == 3))
                if pend_fc:
                    emit_fc(*pend_fc.pop(0))

                # epilogue in halves; chain: r -> t -> u -> tanh;
                # h = n + z*(h_prev - n)  (3 ops after tanh/z)
                hnew = sp.tile([64, H], bf16, tag="h_b")
                ptr = ptt.tile([128, 256], bf16, tag="ptt")
                dst = hTblk[:, i * 256 : (i + 1) * 256]
                for hf in range(2):
                    sl = slice(hf * 256, (hf + 1) * 256)
                    r_ = gp.tile([64, 256], bf16, tag=f"r{hf}")
                    nc.scalar.activation(r_[:], rz_r[:, sl], AF.Sigmoid)
                    t_ = gp.tile([64, 256], bf16, tag=f"t{hf}")
                    nc.vector.tensor_mul(t_[:], r_[:], nh[:, sl])
                    u_ = gp.tile([64, 256], bf16, tag=f"u{hf}")
                    nc.vector.tensor_add(u_[:], t_[:], ni[:, sl])
                    n_ = gp.tile([64, 256], bf16, tag=f"n{hf}")
                    nc.scalar.activation(n_[:], u_[:], AF.Tanh)
                    z_ = gp.tile([64, 256], bf16, tag=f"z{hf}")
                    nc.scalar.activation(z_[:], rz_z[:, sl], AF.Sigmoid)
                    d_ = gp.tile([64, 256], bf16, tag=f"d{hf}")
                    nc.vector.tensor_sub(d_[:], h_b[:, sl], n_[:])
                    zd = gp.tile([64, 256], bf16, tag=f"zd{hf}")
                    nc.vector.tensor_mul(zd[:], z_[:], d_[:])
                    hh_ = hnew[:, sl]
                    nc.vector.tensor_add(hh_, n_[:], zd[:])
                    for c in (2 * hf, 2 * hf + 1):
                        nc.tensor.transpose(ptr[:, c * 64 : (c + 1) * 64],
                                            hnew[:, c * 128 : (c + 1) * 128],
                                            ident[:])
                    cp = (nc.vector.tensor_copy if hf == 0 else
                          (lambda o, i_: nc.scalar.activation(o, i_, AF.Copy)))
                    cp(dst[:, hf * 128 : (hf + 1) * 128],
                       ptr[:, hf * 128 : (hf + 1) * 128])
                h_b = hnew
                if gps is not None:
                    nc.vector.tensor_copy(gi1sl_out_3d[:, i, :], gps)

                pend_fc.append((hTblk, i, ablk, i))

            def emit_bio(q, ablk):
                # ablk of phase q == B's block q-LAG
                # batched a-only terms for the whole block
                a0 = ablk[:, :, :, 0]
                a1 = ablk[:, :, :, 1]
                s_ = bp.tile([64, S, J], f32, tag="s_")
                nc.vector.tensor_add(s_[:], a1, a0)
                dd = bp.tile([64, S, J], f32, tag="dd")
                nc.vector.tensor_sub(dd[:], a1, a0)
                p_ = bp.tile([64, S, J], f32, tag="p_")
                nc.vector.tensor_mul(p_[:], s_[:], dd[:])
                w_ = bp.tile([64, S, J], f32, tag="w_")
                nc.vector.tensor_scalar_mul(w_[:], dd[:], BETA / GAMMA)
                nc.vector.tensor_add(w_[:], w_[:], p_[:])
                q2b = bp.tile([64, S, J], f32, tag="q2b")
                nc.vector.tensor_scalar_mul(q2b[:], w_[:], GAMMA)
                v_ = bp.tile([64, S, J], f32, tag="v_")
                nc.vector.tensor_scalar_mul(v_[:], s_[:], EPS)
                nc.vector.tensor_scalar_add(v_[:], v_[:], DELTA)
                # sequential 2-state chain on Pool
                nonlocal th_ap, om_ap
                g = nc.gpsimd
                for i in range(S):
                    u2 = bp.tile([64, J], f32, tag="u2", name="u2")
                    g.tensor_mul(u2[:], v_[:, i, :], th_ap)
                    q_ = bp.tile([64, J], f32, tag="q_", name="q_")
                    g.tensor_add(q_[:], q2b[:, i, :], u2[:])
                    omA = bp.tile([64, J], f32, tag="omA", name="omA")
                    g.tensor_mul(omA[:], om_ap, cON["ALP"][:])
                    om_new = sp.tile([64, J], f32, tag="om", name="om_new")
                    g.tensor_add(om_new[:], omA[:], q_[:])
                    om_ap = om_new[:]
                    thD = bp.tile([64, J], f32, tag="thD", name="thD")
                    g.tensor_mul(thD[:], om_new[:], cON["DT"][:])
                    if q >= LAG:
                        tout = out_sb[:, ((q - LAG) * S + i) * J :
                                      ((q - LAG) * S + i + 1) * J]
                    else:
                        tout = scratch_th[:, i * J : (i + 1) * J]
                    g.tensor_add(tout, thD[:], th_ap)
                    th_ap = tout

            # =================== phase loop ==============================
            for p in range(NTOT):
                if p < NPH:
                    xts_new = xp.tile([128, 64 * S], bf16, tag="xts")
                    nc.sync.dma_start(xts_new[:], xTp_d[p])
                    xts = xts_new

                hTblk = hbp.tile([128, 256 * S], bf16, tag="hTblk")
                ablk = apl.tile([64, S, J, 2], f32, tag="ablk")
                do_tail = 2 <= p <= NPH + 1
                for i in range(S):
                    emit_step(p, i, hTblk, ablk, do_tail)
                if p == NTOT - 1:
                    while pend_fc:
                        emit_fc(*pend_fc.pop(0))
                # bio for the previous phase's a-block (its last FC lands
                # at step 0 of this phase)
                if prev_ablk is not None:
                    emit_bio(p - 1, prev_ablk)
                if p - 1 == LAG - 1:
                    # switch the bio chain to freshly reset state before the
                    # first real block (processed next phase)
                    th_t2 = sp.tile([64, J], f32, tag="th", name="th_r")
                    nc.sync.dma_start(th_t2[:], th0_d[:])
                    om_t2 = sp.tile([64, J], f32, tag="om", name="om_r")
                    nc.sync.dma_start(om_t2[:], om0_d[:])
                    th_ap = th_t2[:]
                    om_ap = om_t2[:]
                if p == NTOT - 1:
                    emit_bio(p, ablk)

                # ---- collectives ------------------------------------
                if p <= NPH - 1:
                    cc1_in = dp.tile([128, 256 * S], bf16, tag="cc1_in", bufs=2)
                    nc.sync.dma_start(cc1_in[:], hTblk[:])
                    cc1_out = dp.tile([8 * 128, 256 * S], bf16,
                                      addr_space="Shared",
                                      name=f"cc1o_{p}", tag=f"cc1o_{p}")
                    nc.gpsimd.collective_compute(
                        "AllGather", mybir.AluOpType.bypass,
                        ins=[cc1_in[:].opt()], outs=[cc1_out[:].opt()],
                        replica_groups=RG)
                else:
                    cc1_out = None
                if do_tail:
                    cc2_in = dp.tile([64, S, SL], bf16, tag="cc2_in", bufs=2)
                    nc.sync.dma_start(cc2_in[:], gi1sl_out_3d[:])
                    cc2_out = dp.tile([8 * 64, S, SL], bf16,
                                      addr_space="Shared",
                                      name=f"cc2o_{p}", tag=f"cc2o_{p}")
                    nc.gpsimd.collective_compute(
                        "AllGather", mybir.AluOpType.bypass,
                        ins=[cc2_in[:].opt()], outs=[cc2_out[:].opt()],
                        replica_groups=RG)
                else:
                    cc2_out = None

                # ---- fills for next phase ---------------------------
                if 1 <= p <= NPH and prev_cc1_out is not None:
                    nc.sync.dma_start(h0T_recv[:], prev_cc1_out[0:128, :])
                if p in (0, 1, 2):
                    nc.sync.dma_start(gi1buf[0:64, :, :], gi1init_d[:])
                elif prev_cc2_out is not None and p <= NPH + 2:
                    for r in range(4):
                        nc.sync.dma_start(
                            gi1buf[0:64, :, r * SL : (r + 1) * SL],
                            prev_cc2_out[r * 64 : (r + 1) * 64, :, :])

                prev_hTblk = hTblk
                prev_ablk = ablk
                prev_cc1_out = cc1_out
                prev_cc2_out = cc2_out

            nc.sync.dma_start(out_d[:], out_sb[:])
    return nc


_NC_CACHE = {}


def _get_nc(T_run, S=16):
    key = (T_run, S)
    if key in _NC_CACHE:
        return _NC_CACHE[key]
    from concourse import bass, bacc, tile

    mybir = bass.mybir
    nc = bacc.Bacc(None, target_bir_lowering=False, num_devices=8)
    _build(nc, bass, tile, mybir, T_run, S=S)
    nc.compile()
    _NC_CACHE[key] = nc
    return nc


def _prep_inputs(x, W_ih0, W_hh0, b_ih0, b_hh0, W_ih1, W_hh1, b_ih1, b_hh1,
                 fc_W, fc_b, h0, theta0, omega0, S=16):
    """Build the 8 per-core input maps."""
    import ml_dtypes

    bf = ml_dtypes.bfloat16
    f = np.float32
    T_run = x.shape[1]
    NPH = T_run // S

    def bfc(a):
        return np.ascontiguousarray(a).astype(bf)

    maps = []
    # xTp: [NPH, 128, 64*S]; rows 0-15 = x^T, row 16 = ones
    xT_A = np.zeros((NPH, 128, S, 64), f)
    xt = x.transpose(2, 1, 0)  # [IN, T, B]
    xT_A[:, :IN] = xt.reshape(IN, NPH, S, 64).transpose(1, 0, 2, 3)
    xT_A[:, IN] = 1.0
    xT_B = np.zeros((NPH, 128, S, 64), f)
    xT_B[:, IN] = 1.0

    # whh5 per core
    def whh5_for(Whh, Wih, bih, bhh, with_ih):
        out = np.zeros((6, 128, HG), f)
        out[:4] = Whh.T.reshape(4, 128, HG)
        if with_ih:
            out[4, :IN, :] = Wih.T
        brow = np.zeros(HG, f)
        brow[:1024] = (bih + bhh)[:1024]
        brow[1024:] = bih[1024:]
        out[4, IN, :] = brow
        out[5, IN, 1024:] = bhh[1024:]  # b_hh_n panel (ones-row matmul)
        return out

    whh5_A = whh5_for(W_hh0, W_ih0, b_ih0, b_hh0, True)
    whh5_B = whh5_for(W_hh1, None, b_ih1, b_hh1, False)
    whh5_Z = np.zeros((6, 128, HG), f)

    wfc5_B = np.zeros((5, 128, 2 * J), f)
    wfc5_B[:4] = fc_W.T.reshape(4, 128, 2 * J)
    wfc5_B[4, IN, :] = fc_b
    wfc5_Z = np.zeros((5, 128, 2 * J), f)

    selgi_B = np.zeros((128, 64), f)
    selgi_B[:64, :64] = np.eye(64)
    selgi_Z = np.zeros((128, 64), f)

    gi1init_B = np.zeros((S, HG), f)
    gi1init_B[:, 512:1024] = FREEZE
    gi1init_B = np.tile(gi1init_B.reshape(1, S, HG), (64, 1, 1))
    gi1init_Z = np.zeros((64, S, HG), f)

    bhn_A = np.tile(b_hh0[1024:][None, :], (64, 1))
    bhn_B = np.tile(b_hh1[1024:][None, :], (64, 1))
    bhn_Z = np.zeros((64, H), f)

    def hT_of(h):
        o = np.zeros((128, 256), f)
        ht = h.T.reshape(4, 128, 64)  # chunk c, row, batch
        for c in range(4):
            o[:, c * 64 : (c + 1) * 64] = ht[c]
        return o

    for r in range(NC):
        rr = r % 4
        wsl_r = np.ascontiguousarray(
            W_ih1.T[:, rr * SL : (rr + 1) * SL]).reshape(4, 128, SL)
        m = {
            "wsl": bfc(wsl_r),
            "ident": np.eye(64, dtype=f).astype(bf),
            "th0": theta0.astype(f),
            "om0": omega0.astype(f),
        }
        if r == 0:
            m["whh5"] = bfc(whh5_A)
            m["xTp"] = bfc(xT_A.reshape(NPH, 128, S * 64))
            m["wfc5"] = bfc(wfc5_Z)
            m["selgi"] = bfc(selgi_Z)
            m["gi1init"] = bfc(gi1init_Z)
            m["bhn"] = bfc(bhn_A)
            m["hb0"] = bfc(h0[0])
            m["hT0"] = bfc(hT_of(h0[0]))
        elif r == 1:
            m["whh5"] = bfc(whh5_B)
            m["xTp"] = bfc(xT_B.reshape(NPH, 128, S * 64))
            m["wfc5"] = bfc(wfc5_B)
            m["selgi"] = bfc(selgi_B)
            m["gi1init"] = bfc(gi1init_B)
            m["bhn"] = bfc(bhn_B)
            m["hb0"] = bfc(h0[1])
            m["hT0"] = bfc(hT_of(h0[1]))
        else:
            m["whh5"] = bfc(whh5_Z)
            m["xTp"] = bfc(xT_B.reshape(NPH, 128, S * 64))
            m["wfc5"] = bfc(wfc5_Z)
            m["selgi"] = bfc(selgi_Z)
            m["gi1init"] = bfc(gi1init_Z)
            m["bhn"] = bfc(bhn_Z)
            m["hb0"] = bfc(np.zeros((64, H), f))
            m["hT0"] = bfc(np.zeros((128, 256), f))
        maps.append(m)
    return maps


def _install_loud_hook():
    import traceback

    from concourse import bass2jax

    if getattr(bass2jax, "_loud_hook_installed", False):
        return
    orig = bass2jax.neuronx_cc_hook

    def loud(*a, **k):
        try:
            return orig(*a, **k)
        except BaseException:
            traceback.print_exc()
            raise

    bass2jax.neuronx_cc_hook = loud
    bass2jax._loud_hook_installed = True

    # walrus LDWEIGHTS-dedup pass (optional; known to crash codegen in
    # some configurations)
    import os

    from concourse import bass_utils as _bu

    if os.environ.get("KERNEL_LDW_OPT", "0") == "1" and not getattr(
            _bu, "_ldw_patch", False):
        _orig_rc = _bu.run_command

        def _rc(cmd, **kw):
            cmd = [c.replace("--enable-ldw-opt=false", "--enable-ldw-opt=true")
                   if isinstance(c, str) else c for c in cmd]
            return _orig_rc(cmd, **kw)

        _bu.run_command = _rc
        _bu._ldw_patch = True


def run(inputs, S=16, **spmd_kwargs):
    from concourse.bass_utils import run_bass_kernel_spmd

    _install_loud_hook()
    inputs = {k: np.asarray(v) for k, v in inputs.items()}
    T_run = inputs["x"].shape[1]
    nc = _get_nc(T_run, S)
    in_maps = _prep_inputs(**inputs, S=S)
    res = run_bass_kernel_spmd(nc, in_maps, core_ids=list(range(8)),
                               **spmd_kwargs)
    out = res.results[1]["out"].reshape(B, T_run, J).astype(np.float32)
    return out, res


def kernel(**inputs):
    return run(inputs)[0]
